# revision 1
# baseline (speedup 1.0000x reference)
"""Trainium2 Bass kernel for MeshGenLoss (Chamfer + KL + density-uniformity).

Math:
  d[i,j] = |a_i|^2 + |b_j|^2 - 2 a_i.b_j  is computed as ONE K=33 bf16 matmul
  per [128,512] tile: every fp32 scalar is split into 3 exact bf16 limbs, so
  all 9 limb-products of a.b (plus 3 |a|^2 rows against ones, 3 |b|^2 rows)
  accumulate in fp32 PSUM -> fp32-exact distances at bf16 matmul speed.

  Row-min over 4096 cols: ScalarE evacuates PSUM chunks to bf16 SBUF (with
  a free cast), VectorE runs a bf16 min-tree (2x DVE mode) + final
  reduce-min; job flavors A/B trade ScalarE copies vs direct-PSUM VectorE
  reads to balance the two engines.

Sharding: core c owns rows [512c, 512c+512) of each distance matrix
  (pred->target, target->pred, pred->pred self) for both batches = 24 jobs
  of [128 rows x 4096 cols]. For the self matrix the columns are pre-rotated
  by 512c on the host so the masked diagonal always falls in column-tile 0
  (keeps the SPMD program identical across cores); 1e6*I is added there.
"""

import sys

import ml_dtypes
import numpy as np

sys.path.insert(0, "/opt/trn_rl_repo")

B = 2
N = 4096
L = 512
CORES = 8
ROWS = N // CORES  # 512 rows per core
RB = ROWS // 128  # 4 row blocks per core
CT = N // 512  # 8 column tiles per job
K = 33
BF16 = ml_dtypes.bfloat16
BIG = 3.0e38


def _limbs3(x):
    """Split float64 array into 3 bf16 limbs capturing ~24 significand bits."""
    h = x.astype(BF16)
    r = x - h.astype(np.float64)
    m = r.astype(BF16)
    r2 = r - m.astype(np.float64)
    lo = r2.astype(BF16)
    return h, m, lo


def _build_lhsT(a):
    """a: [n, 3] float64 row points -> lhsT [33, n] bf16.

    Rows 0..26: k=(t,p,q) -> -2 * limb_p(a[:, t])  (repeated over q)
    Rows 27..29: limbs of |a|^2
    Rows 30..32: ones (partner of the |b|^2 rhs rows)
    """
    n = a.shape[0]
    asq = (a * a).sum(-1)
    al = _limbs3(a)  # tuple of [n,3] bf16
    sl = _limbs3(asq)
    out = np.zeros((K, n), dtype=BF16)
    k = 0
    for t in range(3):
        for p in range(3):
            row = (-2.0 * al[p][:, t].astype(np.float64)).astype(BF16)
            for _q in range(3):
                out[k] = row
                k += 1
    for p in range(3):
        out[k] = sl[p]
        k += 1
    for _q in range(3):
        out[k] = np.ones(n, dtype=BF16)
        k += 1
    return out


def _build_rhs(b):
    """b: [m, 3] float64 column points -> rhs [33, m] bf16.

    Rows 0..26: k=(t,p,q) -> limb_q(b[:, t])  (repeated over p)
    Rows 27..29: ones (partner of the |a|^2 lhsT rows)
    Rows 30..32: limbs of |b|^2
    """
    m = b.shape[0]
    bsq = (b * b).sum(-1)
    bl = _limbs3(b)
    sl = _limbs3(bsq)
    out = np.zeros((K, m), dtype=BF16)
    k = 0
    for t in range(3):
        for _p in range(3):
            for q in range(3):
                out[k] = bl[q][:, t]
                k += 1
    for _p in range(3):
        out[k] = np.ones(m, dtype=BF16)
        k += 1
    for q in range(3):
        out[k] = sl[q]
        k += 1
    return out


def _build_program():
    import concourse.bacc as bacc
    import concourse.mybir as mybir
    import concourse.tile as tile
    from contextlib import ExitStack

    dt = mybir.dt
    Alu = mybir.AluOpType
    Act = mybir.ActivationFunctionType

    nc = bacc.Bacc("TRN2", target_bir_lowering=False, debug=False)

    d_lhsT_pt = nc.declare_dram_parameter("lhsT_pt", [B, K, ROWS], dt.bfloat16, isOutput=False)
    d_lhsT_tp = nc.declare_dram_parameter("lhsT_tp", [B, K, ROWS], dt.bfloat16, isOutput=False)
    d_rhs_t = nc.declare_dram_parameter("rhs_t", [B, K, N], dt.bfloat16, isOutput=False)
    d_rhs_p = nc.declare_dram_parameter("rhs_p", [B, K, N], dt.bfloat16, isOutput=False)
    d_diag = nc.declare_dram_parameter("diag", [128, 128], dt.float32, isOutput=False)
    d_mu = nc.declare_dram_parameter("mu_sl", [1, 128], dt.float32, isOutput=False)
    d_lv = nc.declare_dram_parameter("lv_sl", [1, 128], dt.float32, isOutput=False)

    o_pt = nc.declare_dram_parameter("o_pt", [B, RB, 128], dt.float32, isOutput=True)
    o_tp = nc.declare_dram_parameter("o_tp", [B, RB, 128], dt.float32, isOutput=True)
    o_pp = nc.declare_dram_parameter("o_pp", [B, RB, 128], dt.float32, isOutput=True)
    o_kl = nc.declare_dram_parameter("o_kl", [1, 3], dt.float32, isOutput=True)
    o_map = {"pt": o_pt, "tp": o_tp, "pp": o_pp}

    with tile.TileContext(nc) as tc, ExitStack() as ctx:
        consts = ctx.enter_context(tc.tile_pool(name="consts", bufs=1))
        psum = ctx.enter_context(tc.tile_pool(name="psum", bufs=4, space="PSUM"))
        cpool = ctx.enter_context(tc.tile_pool(name="cp", bufs=10))
        apool = ctx.enter_context(tc.tile_pool(name="acc", bufs=24))

        # ---- resident inputs (DMA'd in job-consumption order) --------
        lhsT_sb = {}
        rhs_sb = {}
        def load_rhs(dram, b, tag):
            # leading slice first so the first job's matmuls start ~1.5us
            # earlier; remainder streams behind it
            t = consts.tile([K, N], dt.bfloat16, tag=tag)
            nc.sync.dma_start(out=t[:, :1024], in_=dram[b, :, :1024])
            nc.sync.dma_start(out=t[:, 1024:], in_=dram[b, :, 1024:])
            return t

        for b in range(B):
            t1 = consts.tile([K, ROWS], dt.bfloat16, tag=f"lpt{b}")
            nc.sync.dma_start(out=t1[:], in_=d_lhsT_pt[b])
            lhsT_sb["pt", b] = t1
            lhsT_sb["pp", b] = t1
            rhs_sb["pt", b] = load_rhs(d_rhs_t, b, f"rt{b}")
        for b in range(B):
            t2 = consts.tile([K, ROWS], dt.bfloat16, tag=f"ltp{b}")
            nc.sync.dma_start(out=t2[:], in_=d_lhsT_tp[b])
            lhsT_sb["tp", b] = t2
            r2 = load_rhs(d_rhs_p, b, f"rp{b}")
            rhs_sb["tp", b] = r2
            rhs_sb["pp", b] = r2
        diag_sb = consts.tile([128, 128], dt.float32, tag="diag")
        nc.sync.dma_start(out=diag_sb[:], in_=d_diag[:])
        mu_sb = consts.tile([1, 128], dt.float32, tag="mu")
        nc.sync.dma_start(out=mu_sb[:], in_=d_mu[:])
        lv_sb = consts.tile([1, 128], dt.float32, tag="lv")
        nc.sync.dma_start(out=lv_sb[:], in_=d_lv[:])

        # ---- 24 distance-matrix jobs ---------------------------------
        # Two job flavors balance DVE vs ACT:
        #  A: ScalarE copies all 4 PSUM chunks to bf16 SBUF; VectorE does a
        #     pure-bf16 min tree (2x DVE mode).
        #  B: ScalarE copies only odd chunks; VectorE's level-0 mins read
        #     even chunks straight from PSUM (1x).
        # Jobs grouped by (kind, batch) so early jobs only need the rhs
        # tensor that was DMA'd first.
        jobs = [(b, r, kind) for kind in ("pt", "tp", "pp")
                for b in range(B) for r in range(RB)]
        for jidx, (b, r, kind) in enumerate(jobs):
            lhsT = lhsT_sb[kind, b][:, 128 * r:128 * (r + 1)]
            rhs = rhs_sb[kind, b]
            chunks = []
            for h in range(4):
                ch = psum.tile([128, 1024], dt.float32, tag="ps")
                for t in range(2):
                    nc.tensor.matmul(
                        ch[:, 512 * t:512 * (t + 1)],
                        lhsT, rhs[:, 1024 * h + 512 * t:1024 * h + 512 * (t + 1)],
                        start=True, stop=True,
                    )
                chunks.append(ch)
            if kind == "pp":
                # mask the self-distance diagonal (always in chunk 0 at
                # offset 128*r thanks to the host-side column rotation)
                sl = chunks[0][:, 128 * r:128 * r + 128]
                nc.vector.tensor_tensor(sl, sl, diag_sb[:], Alu.add)
            # first jobs are B-type so VectorE starts after a single copy;
            # none of the DVE-heavy B-jobs in the last stretch
            a_type = jidx not in (0, 5, 7, 9, 11, 13, 17, 19, 21, 23)  # 14 of 24
            if a_type:
                # all 4 chunks into one contiguous bf16 staging buffer ->
                # the whole tree runs as in-place halving on wide 2x TTs
                st = cpool.tile([128, 4096], dt.bfloat16, tag="cp4", bufs=3)
                for h in range(4):
                    nc.scalar.copy(st[:, 1024 * h:1024 * (h + 1)], chunks[h][:])
                nc.vector.tensor_tensor(
                    st[:, :2048], st[:, :2048], st[:, 2048:], Alu.min)
                nc.vector.tensor_tensor(
                    st[:, :1024], st[:, :1024], st[:, 1024:2048], Alu.min)
                m01 = st
            else:
                m01 = cpool.tile([128, 1024], dt.bfloat16, tag="cp")
                m23 = cpool.tile([128, 1024], dt.bfloat16, tag="cp")
                cb1 = cpool.tile([128, 1024], dt.bfloat16, tag="cp")
                nc.scalar.copy(cb1[:], chunks[1][:])
                nc.vector.tensor_tensor(m01[:], chunks[0][:], cb1[:], Alu.min)
                cb3 = cpool.tile([128, 1024], dt.bfloat16, tag="cp")
                nc.scalar.copy(cb3[:], chunks[3][:])
                nc.vector.tensor_tensor(m23[:], chunks[2][:], cb3[:], Alu.min)
                nc.vector.tensor_tensor(m01[:], m01[:], m23[:], Alu.min)
            nc.vector.tensor_tensor(
                m01[:, :512], m01[:, :512], m01[:, 512:1024], Alu.min)
            acc = apool.tile([128, 1], dt.float32, tag="acc")
            nc.vector.tensor_reduce(
                acc[:], m01[:, :512], axis=mybir.AxisListType.X, op=Alu.min)
            nc.sync.dma_start(out=o_map[kind][b, r, :], in_=acc[:, 0])

        # ---- KL partials (at the end: the Exp table-load then overlaps
        # trailing job work instead of delaying the first ACT copies) ----
        s1 = apool.tile([1, 1], dt.float32, tag="kls")
        nc.vector.tensor_reduce(s1[:], lv_sb[:], axis=mybir.AxisListType.X, op=Alu.add)
        e_t = consts.tile([1, 128], dt.float32, tag="klexp")
        s3 = apool.tile([1, 1], dt.float32, tag="kls")
        nc.scalar.activation(e_t[:], lv_sb[:], Act.Exp, accum_out=s3[:])
        sq_t = consts.tile([1, 128], dt.float32, tag="klsq")
        s2 = apool.tile([1, 1], dt.float32, tag="kls")
        nc.scalar.activation(sq_t[:], mu_sb[:], Act.Square, accum_out=s2[:])
        nc.sync.dma_start(out=o_kl[0, 0:1], in_=s1[:, 0])
        nc.sync.dma_start(out=o_kl[0, 1:2], in_=s2[:, 0])
        nc.sync.dma_start(out=o_kl[0, 2:3], in_=s3[:, 0])

    nc.compile()
    return nc


def _make_in_maps(pred, target, mu, logvar):
    pred = np.asarray(pred, dtype=np.float32)
    target = np.asarray(target, dtype=np.float32)
    mu = np.asarray(mu, dtype=np.float32)
    logvar = np.asarray(logvar, dtype=np.float32)

    pred64 = pred.astype(np.float64)
    target64 = target.astype(np.float64)

    # Shared (core-independent) operands
    rhs_t = np.stack([_build_rhs(target64[b]) for b in range(B)])  # [B,K,N]
    rhs_p_full = np.stack([_build_rhs(pred64[b]) for b in range(B)])
    diag = (np.eye(128, dtype=np.float32) * 1.0e6)
    mu_flat = mu.reshape(-1)
    lv_flat = logvar.reshape(-1)

    in_maps = []
    for c in range(CORES):
        rows = slice(ROWS * c, ROWS * (c + 1))
        lhsT_pt = np.stack([_build_lhsT(pred64[b, rows]) for b in range(B)])
        lhsT_tp = np.stack([_build_lhsT(target64[b, rows]) for b in range(B)])
        rot = np.roll(rhs_p_full, -ROWS * c, axis=2)
        in_maps.append({
            "lhsT_pt": lhsT_pt,
            "lhsT_tp": lhsT_tp,
            "rhs_t": rhs_t,
            "rhs_p": np.ascontiguousarray(rot),
            "diag": diag,
            "mu_sl": mu_flat[128 * c:128 * (c + 1)].reshape(1, 128),
            "lv_sl": lv_flat[128 * c:128 * (c + 1)].reshape(1, 128),
        })
    return in_maps


def kernel(pred, target, mu, logvar):
    from concourse.bass_utils import run_bass_kernel_spmd

    in_maps = _make_in_maps(pred, target, mu, logvar)
    nc = _build_program()
    res = run_bass_kernel_spmd(nc, in_maps, list(range(CORES)))
    results = res.results

    nn_pt = np.concatenate([r["o_pt"].reshape(B, ROWS) for r in results], axis=1)
    nn_tp = np.concatenate([r["o_tp"].reshape(B, ROWS) for r in results], axis=1)
    nn_pp = np.concatenate([r["o_pp"].reshape(B, ROWS) for r in results], axis=1)
    kl_parts = np.stack([r["o_kl"].reshape(3) for r in results])  # [CORES,3]

    nn_pt64 = nn_pt.astype(np.float64)
    nn_tp64 = nn_tp.astype(np.float64)
    nn_pp64 = nn_pp.astype(np.float64)

    cd = (nn_pt64.mean(axis=1) + nn_tp64.mean(axis=1)).mean()

    s1 = kl_parts[:, 0].astype(np.float64).sum()
    s2 = kl_parts[:, 1].astype(np.float64).sum()
    s3 = kl_parts[:, 2].astype(np.float64).sum()
    n_kl = B * L
    kl = -0.5 * (n_kl + s1 - s2 - s3) / n_kl

    density = np.std(nn_pp64, axis=1, ddof=1).mean()

    total = cd + 0.001 * kl + 0.1 * density

    return (
        np.float32(total),
        np.float32(cd),
        np.float32(kl),
        np.float32(density),
    )



# revision 7
# speedup vs baseline: 1.2289x; 1.2289x over previous
"""Trainium2 Bass kernel for MeshGenLoss (Chamfer + KL + density-uniformity).

Math:
  d[i,j] = |a_i|^2 + |b_j|^2 - 2 a_i.b_j  via bf16 limb matmuls: every fp32
  scalar splits into 3 exact bf16 limbs; keeping products with p+q <= 2 plus
  3-limb norms gives K=24 contraction rows (~2^-23 dot error, fp32 PSUM).

PE 4-way row packing (v7): K=24 <= 32, so the 128x128 PE array is split into
  four 32-row groups via tile_position=(32g, 0).  The four matmuls of a
  "quad" hold the same weights in their own row group, stream four different
  512-column slices concurrently (measured cadence ~65ns per 512-col matmul
  vs 439ns unpacked; the PE in this environment is locked at 1.2 GHz), and
  write adjacent PSUM slices.  Host packs lhsT/rhs so group g sits on SBUF
  partitions 32g..32g+23.

Row-min evacuation: two job flavors balance ScalarE vs VectorE (the only
  PSUM-capable / min-capable engines; Pool cannot touch PSUM nor run min,
  tensor_tensor_reduce hard-faults the device, tensor_tensor_scan is
  2 cycles/element):
   A-flavor: ScalarE copies all 4 chunks into an fp16 staging tile; VectorE
     runs an all-2x in-place halving tree down to [128,512].
   D-flavor: ScalarE copies chunks 0,2; VectorE pairs chunks 1,3 against the
     seeds with 1x tensor_tensors, then a 2x tree down to [128,512].
  The [128,512] fp16 tile DMAs out per job; the HOST does the final 512-way
  min (saves the 1x on-chip reduce).  The pp self-distance diagonal is masked
  by a Pool-engine ADD of 6e4 onto the fp16 seed copy (Pool supports add on
  SBUF), so VectorE never touches it.

Sharding: core c owns rows [512c, 512c+512) of each distance matrix
  (pred->target, target->pred, pred->pred self) for both batches = 24 jobs
  of [128 rows x 4096 cols]. For the self matrix the columns are pre-rotated
  by 512c on the host so the masked diagonal always falls in column-tile 0
  (keeps the SPMD program identical across cores).
"""

import sys

import ml_dtypes
import numpy as np

sys.path.insert(0, "/opt/trn_rl_repo")

B = 2
N = 4096
L = 512
CORES = 8
ROWS = N // CORES  # 512 rows per core
RB = ROWS // 128  # 4 row blocks per core
K = 24
BF16 = ml_dtypes.bfloat16
FP16 = np.float16
BIG = 3.0e38
DIAG = 6.0e4  # > max squared distance (~50), fp16-safe

# limb-product pairs (p from lhs, q from rhs), p+q <= 2
PAIRS = [(0, 0), (0, 1), (1, 0), (1, 1), (0, 2), (2, 0)]


def _limbs3(x):
    """Split float64 array into 3 bf16 limbs capturing ~24 significand bits."""
    h = x.astype(BF16)
    r = x - h.astype(np.float64)
    m = r.astype(BF16)
    r2 = r - m.astype(np.float64)
    lo = r2.astype(BF16)
    return h, m, lo


def _build_lhsT(a):
    """a: [n, 3] float64 row points -> lhsT [24, n] bf16.

    Rows 0..17: (t, (p,q)) -> -2 * limb_p(a[:, t])
    Rows 18..20: limbs of |a|^2     (partner: ones)
    Rows 21..23: ones               (partner: limbs of |b|^2)
    """
    n = a.shape[0]
    asq = (a * a).sum(-1)
    al = _limbs3(a)
    sl = _limbs3(asq)
    out = np.zeros((K, n), dtype=BF16)
    k = 0
    for t in range(3):
        for p, _q in PAIRS:
            out[k] = (-2.0 * al[p][:, t].astype(np.float64)).astype(BF16)
            k += 1
    for p in range(3):
        out[k] = sl[p]
        k += 1
    for _q in range(3):
        out[k] = np.ones(n, dtype=BF16)
        k += 1
    return out


def _build_rhs(b):
    """b: [m, 3] float64 column points -> rhs [24, m] bf16."""
    m = b.shape[0]
    bsq = (b * b).sum(-1)
    bl = _limbs3(b)
    sl = _limbs3(bsq)
    out = np.zeros((K, m), dtype=BF16)
    k = 0
    for t in range(3):
        for _p, q in PAIRS:
            out[k] = bl[q][:, t]
            k += 1
    for _p in range(3):
        out[k] = np.ones(m, dtype=BF16)
        k += 1
    for q in range(3):
        out[k] = sl[q]
        k += 1
    return out


def _pack_groups(x, free_pack):
    """x: [K, F] -> packed [128, F'] with group g on partitions 32g..32g+K-1.

    free_pack: if True, F = 4096 original columns are split so group g holds
    columns 2048u + 512g + j at packed column 512u + j (u = quad index).
    If False (weights), every group holds the same F columns.
    """
    if free_pack:
        F = x.shape[1]
        nq = F // 2048
        out = np.zeros((128, 512 * nq), dtype=x.dtype)
        for g in range(4):
            for u in range(nq):
                out[32 * g:32 * g + K, 512 * u:512 * (u + 1)] = \
                    x[:, 2048 * u + 512 * g:2048 * u + 512 * g + 512]
    else:
        out = np.zeros((128, x.shape[1]), dtype=x.dtype)
        for g in range(4):
            out[32 * g:32 * g + K] = x
    return out


def _build_program():
    import concourse.bacc as bacc
    import concourse.mybir as mybir
    import concourse.tile as tile
    from contextlib import ExitStack

    dt = mybir.dt
    Alu = mybir.AluOpType
    Act = mybir.ActivationFunctionType

    nc = bacc.Bacc("TRN2", target_bir_lowering=False, debug=False)

    d_lhsT_pt = nc.declare_dram_parameter("lhsT_pt", [B, 128, ROWS], dt.bfloat16, isOutput=False)
    d_lhsT_tp = nc.declare_dram_parameter("lhsT_tp", [B, 128, ROWS], dt.bfloat16, isOutput=False)
    d_rhs_t = nc.declare_dram_parameter("rhs_t", [B, 128, N // 4], dt.bfloat16, isOutput=False)
    d_rhs_p = nc.declare_dram_parameter("rhs_p", [B, 128, N // 4], dt.bfloat16, isOutput=False)
    d_diag = nc.declare_dram_parameter("diag", [128, 128], dt.float16, isOutput=False)
    d_mu = nc.declare_dram_parameter("mu_sl", [1, 128], dt.float32, isOutput=False)
    d_lv = nc.declare_dram_parameter("lv_sl", [1, 128], dt.float32, isOutput=False)

    o_pt = nc.declare_dram_parameter("o_pt", [B, RB, 128, 512], dt.float16, isOutput=True)
    o_tp = nc.declare_dram_parameter("o_tp", [B, RB, 128, 512], dt.float16, isOutput=True)
    o_pp = nc.declare_dram_parameter("o_pp", [B, RB, 128, 512], dt.float16, isOutput=True)
    o_kl = nc.declare_dram_parameter("o_kl", [1, 3], dt.float32, isOutput=True)
    o_map = {"pt": o_pt, "tp": o_tp, "pp": o_pp}

    with tile.TileContext(nc) as tc, ExitStack() as ctx:
        consts = ctx.enter_context(tc.tile_pool(name="consts", bufs=1))
        psum = ctx.enter_context(tc.tile_pool(name="psum", bufs=4, space="PSUM"))
        seedp = ctx.enter_context(tc.tile_pool(name="seedp", bufs=4))
        junkp = ctx.enter_context(tc.tile_pool(name="junkp", bufs=6))
        apool = ctx.enter_context(tc.tile_pool(name="acc", bufs=8))

        # ---- resident inputs (DMA'd in job-consumption order) --------
        lhsT_sb = {}
        rhs_sb = {}
        def load_rhs(dram, b, tag):
            t = consts.tile([128, N // 4], dt.bfloat16, tag=tag)
            nc.sync.dma_start(out=t[:, :256], in_=dram[b, :, :256])
            nc.sync.dma_start(out=t[:, 256:], in_=dram[b, :, 256:])
            return t

        for b in range(B):
            t1 = consts.tile([128, ROWS], dt.bfloat16, tag=f"lpt{b}")
            nc.sync.dma_start(out=t1[:], in_=d_lhsT_pt[b])
            lhsT_sb["pt", b] = t1
            lhsT_sb["pp", b] = t1
            rhs_sb["pt", b] = load_rhs(d_rhs_t, b, f"rt{b}")
        for b in range(B):
            t2 = consts.tile([128, ROWS], dt.bfloat16, tag=f"ltp{b}")
            nc.sync.dma_start(out=t2[:], in_=d_lhsT_tp[b])
            lhsT_sb["tp", b] = t2
            r2 = load_rhs(d_rhs_p, b, f"rp{b}")
            rhs_sb["tp", b] = r2
            rhs_sb["pp", b] = r2
        diag_sb = consts.tile([128, 128], dt.float16, tag="diag")
        nc.sync.dma_start(out=diag_sb[:], in_=d_diag[:])
        mu_sb = consts.tile([1, 128], dt.float32, tag="mu")
        nc.sync.dma_start(out=mu_sb[:], in_=d_mu[:])
        lv_sb = consts.tile([1, 128], dt.float32, tag="lv")
        nc.sync.dma_start(out=lv_sb[:], in_=d_lv[:])

        # ---- 24 distance-matrix jobs ---------------------------------
        # 8 A-flavor / 16 D-flavor interleaved balances ACT vs DVE.
        a_set = {1, 4, 7, 10, 13, 16, 19, 22}
        jobs = [(b, r, kind) for kind in ("pt", "tp", "pp")
                for b in range(B) for r in range(RB)]
        for jidx, (b, r, kind) in enumerate(jobs):
            lhsT = lhsT_sb[kind, b]
            rhs = rhs_sb[kind, b]
            chunks = []
            for u in range(2):  # quads: each fills two [128,1024] chunks
                ca = psum.tile([128, 1024], dt.float32, tag="ps")
                cb = psum.tile([128, 1024], dt.float32, tag="ps")
                outs = [ca[:, :512], ca[:, 512:], cb[:, :512], cb[:, 512:]]
                for g in range(4):
                    nc.tensor.matmul(
                        outs[g],
                        lhsT[32 * g:32 * g + K, 128 * r:128 * (r + 1)],
                        rhs[32 * g:32 * g + K, 512 * u:512 * (u + 1)],
                        start=True, stop=True,
                        tile_position=(32 * g, 0),
                    )
                chunks.extend([ca, cb])

            if jidx in a_set:
                st = seedp.tile([128, 4096], dt.float16, tag="st", bufs=3)
                for h in range(4):
                    nc.scalar.copy(st[:, 1024 * h:1024 * (h + 1)], chunks[h][:])
                if kind == "pp":
                    sl = st[:, 128 * r:128 * r + 128]
                    nc.gpsimd.tensor_tensor(sl, sl, diag_sb[:], Alu.add)
                nc.vector.tensor_tensor(
                    st[:, :2048], st[:, :2048], st[:, 2048:], Alu.min)
                nc.vector.tensor_tensor(
                    st[:, :1024], st[:, :1024], st[:, 1024:2048], Alu.min)
                nc.vector.tensor_tensor(
                    st[:, :512], st[:, :512], st[:, 512:1024], Alu.min)
                fout = st[:, :512]
            else:
                s0 = seedp.tile([128, 1024], dt.float16, tag="sd")
                nc.scalar.copy(s0[:], chunks[0][:])
                if kind == "pp":
                    sl = s0[:, 128 * r:128 * r + 128]
                    nc.gpsimd.tensor_tensor(sl, sl, diag_sb[:], Alu.add)
                s2 = seedp.tile([128, 1024], dt.float16, tag="sd")
                nc.scalar.copy(s2[:], chunks[2][:])
                m01 = junkp.tile([128, 1024], dt.float16, tag="jk")
                m23 = junkp.tile([128, 1024], dt.float16, tag="jk")
                nc.vector.tensor_tensor(m01[:], chunks[1][:], s0[:], Alu.min)
                nc.vector.tensor_tensor(m23[:], chunks[3][:], s2[:], Alu.min)
                nc.vector.tensor_tensor(m01[:], m01[:], m23[:], Alu.min)
                nc.vector.tensor_tensor(
                    m01[:, :512], m01[:, :512], m01[:, 512:1024], Alu.min)
                fout = m01[:, :512]
            nc.sync.dma_start(out=o_map[kind][b, r], in_=fout)

        # ---- KL partials ---------------------------------------------
        s1 = apool.tile([1, 1], dt.float32, tag="kls")
        nc.vector.tensor_reduce(s1[:], lv_sb[:], axis=mybir.AxisListType.X, op=Alu.add)
        e_t = consts.tile([1, 128], dt.float32, tag="klexp")
        s3 = apool.tile([1, 1], dt.float32, tag="kls")
        nc.scalar.activation(e_t[:], lv_sb[:], Act.Exp, accum_out=s3[:])
        sq_t = consts.tile([1, 128], dt.float32, tag="klsq")
        s2k = apool.tile([1, 1], dt.float32, tag="kls")
        nc.scalar.activation(sq_t[:], mu_sb[:], Act.Square, accum_out=s2k[:])
        nc.sync.dma_start(out=o_kl[0, 0:1], in_=s1[:, 0])
        nc.sync.dma_start(out=o_kl[0, 1:2], in_=s2k[:, 0])
        nc.sync.dma_start(out=o_kl[0, 2:3], in_=s3[:, 0])

    nc.compile()
    return nc


def _make_in_maps(pred, target, mu, logvar):
    pred = np.asarray(pred, dtype=np.float32)
    target = np.asarray(target, dtype=np.float32)
    mu = np.asarray(mu, dtype=np.float32)
    logvar = np.asarray(logvar, dtype=np.float32)

    pred64 = pred.astype(np.float64)
    target64 = target.astype(np.float64)

    # Shared (core-independent) operands, packed for 4-way PE row tiling
    rhs_t = np.stack([_pack_groups(_build_rhs(target64[b]), True) for b in range(B)])
    rhs_p_full = np.stack([_build_rhs(pred64[b]) for b in range(B)])  # [B,K,N]
    diag = (np.eye(128) * DIAG).astype(np.float16)
    mu_flat = mu.reshape(-1)
    lv_flat = logvar.reshape(-1)

    in_maps = []
    for c in range(CORES):
        rows = slice(ROWS * c, ROWS * (c + 1))
        lhsT_pt = np.stack([_pack_groups(_build_lhsT(pred64[b, rows]), False) for b in range(B)])
        lhsT_tp = np.stack([_pack_groups(_build_lhsT(target64[b, rows]), False) for b in range(B)])
        rot = np.roll(rhs_p_full, -ROWS * c, axis=2)
        rhs_p = np.stack([_pack_groups(rot[b], True) for b in range(B)])
        in_maps.append({
            "lhsT_pt": lhsT_pt,
            "lhsT_tp": lhsT_tp,
            "rhs_t": rhs_t,
            "rhs_p": rhs_p,
            "diag": diag,
            "mu_sl": mu_flat[128 * c:128 * (c + 1)].reshape(1, 128),
            "lv_sl": lv_flat[128 * c:128 * (c + 1)].reshape(1, 128),
        })
    return in_maps


def kernel(pred, target, mu, logvar):
    from concourse.bass_utils import run_bass_kernel_spmd

    in_maps = _make_in_maps(pred, target, mu, logvar)
    nc = _build_program()
    res = run_bass_kernel_spmd(nc, in_maps, list(range(CORES)))
    results = res.results

    def collect(name):
        # [CORES][B, RB, 128, 512] fp16 -> [B, N] rowmin via host min
        per = [np.asarray(r[name], dtype=np.float32).min(axis=-1) for r in results]
        return np.concatenate([p.reshape(B, ROWS) for p in per], axis=1)

    nn_pt = collect("o_pt")
    nn_tp = collect("o_tp")
    nn_pp = collect("o_pp")
    kl_parts = np.stack([r["o_kl"].reshape(3) for r in results])  # [CORES,3]

    nn_pt64 = nn_pt.astype(np.float64)
    nn_tp64 = nn_tp.astype(np.float64)
    nn_pp64 = nn_pp.astype(np.float64)

    cd = (nn_pt64.mean(axis=1) + nn_tp64.mean(axis=1)).mean()

    s1 = kl_parts[:, 0].astype(np.float64).sum()
    s2 = kl_parts[:, 1].astype(np.float64).sum()
    s3 = kl_parts[:, 2].astype(np.float64).sum()
    n_kl = B * L
    kl = -0.5 * (n_kl + s1 - s2 - s3) / n_kl

    density = np.std(nn_pp64, axis=1, ddof=1).mean()

    total = cd + 0.001 * kl + 0.1 * density

    return (
        np.float32(total),
        np.float32(cd),
        np.float32(kl),
        np.float32(density),
    )
